# revision 6
# baseline (speedup 1.0000x reference)
"""GQA kernel for trn2, 8 NeuronCores.

Problem: B=2, S=2048, D=2048, 16 q-heads, 4 KV groups, head_dim=128.
Sharding: one core per (batch, group) pair -> 8 cores.
Per core: q/k/v projections for its group, 4-head attention against the
group's single KV head, partial output projection (Wc row-block). Host
sums the 4 group partials per batch and adds bc.

Layout trick: host feeds q/k/v pre-transposed (d on partitions) so every
matmul operand has its contraction dim on partitions. Scores are computed
transposed (S^T [skv, sq]); softmax uses exp without max subtraction
(scores are O(+-5), safe in fp32), column sums via PE ones-matmul, and
normalization via elementwise multiply with a PE-replicated reciprocal.

Matmuls run in float32r (fp32 with 11-bit mantissa, 4x faster than fp32).
"""

import sys

sys.path.insert(0, "/opt/trn_rl_repo")

import numpy as np

import concourse.bass as bass  # noqa: F401
import concourse.tile as tile
import concourse.mybir as mybir
from concourse import bacc
from concourse.bass_utils import run_bass_kernel_spmd
from concourse.masks import make_identity

F32 = mybir.dt.float32
F32R = mybir.dt.float32r
AF = mybir.ActivationFunctionType

P = 128
D = 2048          # d_model
S = 2048          # sequence length
HG = 4            # query heads per group (per core)
HD = 128          # head dim
H = HG * HD       # 512, per-core q-projection width
B = 2
G = 4
N_CORES = 8
DK = D // P       # 16 contraction tiles
NT = S // P       # 16 seq tiles of 128
NC = S // 512     # 4 seq chunks of 512

_CACHE = {}


def _build():
    nc = bacc.Bacc("TRN2", target_bir_lowering=False, debug=False,
                   num_devices=N_CORES)

    qT = nc.dram_tensor("qT", [D, S], F32R, kind="ExternalInput").ap()
    kT = nc.dram_tensor("kT", [D, S], F32R, kind="ExternalInput").ap()
    vT = nc.dram_tensor("vT", [D, S], F32R, kind="ExternalInput").ap()
    wq = nc.dram_tensor("wq", [D, H], F32R, kind="ExternalInput").ap()
    wk = nc.dram_tensor("wk", [D, HD], F32R, kind="ExternalInput").ap()
    wv = nc.dram_tensor("wv", [D, HD], F32R, kind="ExternalInput").ap()
    wc = nc.dram_tensor("wc", [H, D], F32R, kind="ExternalInput").ap()
    ones_d = nc.dram_tensor("ones", [P, P], F32R, kind="ExternalInput").ap()
    bqp = nc.dram_tensor("bqp", [P, HG], F32, kind="ExternalInput").ap()
    bkp = nc.dram_tensor("bkp", [P, 1], F32, kind="ExternalInput").ap()
    bvp = nc.dram_tensor("bvp", [P, 1], F32, kind="ExternalInput").ap()
    y = nc.dram_tensor("y", [S, D], F32, kind="ExternalOutput").ap()

    with tile.TileContext(nc) as tc:
        # ---- persistent tiles -------------------------------------------
        with tc.tile_pool(name="persist", bufs=1) as persist:
            ones_t = persist.tile([P, P], F32R)
            nc.sync.dma_start(ones_t[:], ones_d[:])
            ident = persist.tile([P, P], F32)
            make_identity(nc, ident[:])
            bq_t = persist.tile([P, HG], F32)
            nc.sync.dma_start(bq_t[:], bqp[:])
            bk_t = persist.tile([P, 1], F32)
            nc.sync.dma_start(bk_t[:], bkp[:])
            bv_t = persist.tile([P, 1], F32)
            nc.sync.dma_start(bv_t[:], bvp[:])

            # wc resident for phase F: 4 tiles [128, D]
            wc_t = []
            for hh in range(HG):
                t = persist.tile([P, D], F32R, tag=f"wc{hh}", name="wc_t")
                nc.sync.dma_start(t[:], wc[hh * P:(hh + 1) * P, :])
                wc_t.append(t)

            # projection outputs (persist through phase D/E)
            qpT = [persist.tile([P, S], F32R, tag=f"qpT{i}", name=f"qpT{i}") for i in range(HG)]
            kpT = persist.tile([P, S], F32R)
            vp = [persist.tile([P, P], F32R, tag=f"vp{i}", name=f"vp{i}") for i in range(NT)]
            otn = [persist.tile([P, S], F32R, tag=f"otn{i}", name=f"otn{i}") for i in range(HG)]

            # ---- phase A/B/C: projections -------------------------------
            with tc.tile_pool(name="wgt", bufs=1) as wgt, \
                 tc.tile_pool(name="xt", bufs=9) as xtp, \
                 tc.tile_pool(name="vpt", bufs=1) as vptp, \
                 tc.tile_pool(name="psA", bufs=1, space="PSUM") as psA, \
                 tc.tile_pool(name="psT", bufs=2, space="PSUM") as psT:
                wq_t = []
                for k in range(DK):
                    t = wgt.tile([P, H], F32R, tag=f"wq{k}", name="wq_t")
                    nc.sync.dma_start(t[:], wq[k * P:(k + 1) * P, :])
                    wq_t.append(t)
                wk_t = []
                for k in range(DK):
                    t = wgt.tile([P, HD], F32R, tag=f"wk{k}", name="wk_t")
                    nc.sync.dma_start(t[:], wk[k * P:(k + 1) * P, :])
                    wk_t.append(t)
                wv_t = []
                for k in range(DK):
                    t = wgt.tile([P, HD], F32R, tag=f"wv{k}", name="wv_t")
                    nc.sync.dma_start(t[:], wv[k * P:(k + 1) * P, :])
                    wv_t.append(t)

                vpT = vptp.tile([P, S], F32)

                for n in range(NC):
                    sl = slice(n * 512, (n + 1) * 512)
                    # six psum accumulators live across the k loop
                    psq = [psA.tile([P, 512], F32, tag=f"psq{m}",
                                    name="psq") for m in range(HG)]
                    psk = psA.tile([P, 512], F32, tag="psk", name="psk")
                    psv = psA.tile([P, 512], F32, tag="psv", name="psv")
                    for k in range(DK):
                        qx = xtp.tile([P, 512], F32R, tag="xt", name="qx")
                        nc.sync.dma_start(qx[:], qT[k * P:(k + 1) * P, sl])
                        kx = xtp.tile([P, 512], F32R, tag="xt", name="kx")
                        nc.sync.dma_start(kx[:], kT[k * P:(k + 1) * P, sl])
                        vx = xtp.tile([P, 512], F32R, tag="xt", name="vx")
                        nc.sync.dma_start(vx[:], vT[k * P:(k + 1) * P, sl])
                        for m in range(HG):
                            nc.tensor.matmul(
                                psq[m][:], wq_t[k][:, m * P:(m + 1) * P],
                                qx[:], start=(k == 0), stop=(k == DK - 1),
                                skip_group_check=True)
                        nc.tensor.matmul(psk[:], wk_t[k][:], kx[:],
                                         start=(k == 0), stop=(k == DK - 1),
                                         skip_group_check=True)
                        nc.tensor.matmul(psv[:], wv_t[k][:], vx[:],
                                         start=(k == 0), stop=(k == DK - 1),
                                         skip_group_check=True)
                    for m in range(HG):
                        nc.scalar.activation(qpT[m][:, sl], psq[m][:],
                                             AF.Identity, bias=bq_t[:, m:m + 1])
                    nc.scalar.activation(kpT[:, sl], psk[:],
                                         AF.Identity, bias=bk_t[:, 0:1])
                    nc.scalar.activation(vpT[:, sl], psv[:],
                                         AF.Identity, bias=bv_t[:, 0:1])

                # vp[t] = vpT[:, t].T  via PE transpose
                for t in range(NT):
                    ps = psT.tile([P, P], F32, tag="pst", name="pst")
                    nc.tensor.transpose(ps[:], vpT[:, t * P:(t + 1) * P],
                                        ident[:])
                    nc.vector.tensor_copy(vp[t][:], ps[:])

            # ---- phase D/E: attention per head --------------------------
            with tc.tile_pool(name="pT", bufs=4) as pTp, \
                 tc.tile_pool(name="rc", bufs=2) as rcp, \
                 tc.tile_pool(name="psS", bufs=2, space="PSUM") as psS, \
                 tc.tile_pool(name="psOT", bufs=1, space="PSUM") as psOT, \
                 tc.tile_pool(name="psCS", bufs=1, space="PSUM") as psCS:
                for h in range(HG):
                    for np_ in range(2):  # s-chunks of 1024
                        base = np_ * 1024
                        ot_ps = psOT.tile([P, 1024], F32, tag="ot", name="ot_ps")
                        cs_ps = psCS.tile([P, 1024], F32, tag="cs", name="cs_ps")
                        for m in range(NT):
                            s_ps = psS.tile([P, 1024], F32, tag="s", name="s_ps")
                            for c in range(2):
                                nc.tensor.matmul(
                                    s_ps[:, c * 512:(c + 1) * 512],
                                    kpT[:, m * P:(m + 1) * P],
                                    qpT[h][:, base + c * 512:base + (c + 1) * 512],
                                    start=True, stop=True,
                                    skip_group_check=True)
                            pt = pTp.tile([P, 1024], F32R, tag="pt", name="pt")
                            nc.scalar.activation(pt[:], s_ps[:], AF.Exp)
                            for c in range(2):
                                cc = slice(c * 512, (c + 1) * 512)
                                nc.tensor.matmul(
                                    cs_ps[:, cc], ones_t[:], pt[:, cc],
                                    start=(m == 0), stop=(m == NT - 1),
                                    skip_group_check=True)
                                nc.tensor.matmul(
                                    ot_ps[:, cc], vp[m][:], pt[:, cc],
                                    start=(m == 0), stop=(m == NT - 1),
                                    skip_group_check=True)
                        rc = rcp.tile([P, 1024], F32, tag="rc", name="rc")
                        nc.vector.reciprocal(rc[:], cs_ps[:])
                        nc.vector.tensor_mul(otn[h][:, base:base + 1024],
                                             ot_ps[:], rc[:])

            # ---- phase F: y = OT.T @ Wc ---------------------------------
            with tc.tile_pool(name="psY", bufs=4, space="PSUM") as psY, \
                 tc.tile_pool(name="yst", bufs=4) as ystp:
                for mt in range(NT):
                    for n in range(NC):
                        ps = psY.tile([P, 512], F32, tag="y", name="psy")
                        for hh in range(HG):
                            nc.tensor.matmul(
                                ps[:], otn[hh][:, mt * P:(mt + 1) * P],
                                wc_t[hh][:, n * 512:(n + 1) * 512],
                                start=(hh == 0), stop=(hh == HG - 1))
                        yt = ystp.tile([P, 512], F32, tag="yt", name="yt")
                        nc.vector.tensor_copy(yt[:], ps[:])
                        nc.sync.dma_start(
                            y[mt * P:(mt + 1) * P, n * 512:(n + 1) * 512],
                            yt[:])

    nc.compile()
    return nc


def _get_nc():
    if "nc" not in _CACHE:
        _CACHE["nc"] = _build()
    return _CACHE["nc"]


def kernel(q, k, v, Wq, bq, Wk, bk, Wv, bv, Wc, bc):
    nc = _get_nc()
    scale = np.float32(1.0 / np.sqrt(HD))
    ones = np.ones((P, P), np.float32)

    in_maps = []
    for b in range(B):
        qTb = np.ascontiguousarray(np.asarray(q[b], np.float32).T)
        kTb = np.ascontiguousarray(np.asarray(k[b], np.float32).T)
        vTb = np.ascontiguousarray(np.asarray(v[b], np.float32).T)
        for g in range(G):
            wq_g = np.ascontiguousarray(
                np.asarray(Wq[:, g * H:(g + 1) * H], np.float32)) * scale
            bq_g = (np.asarray(bq[g * H:(g + 1) * H], np.float32)
                    * scale).reshape(HG, P).T.copy()
            wk_g = np.ascontiguousarray(
                np.asarray(Wk[:, g * HD:(g + 1) * HD], np.float32))
            bk_g = np.asarray(bk[g * HD:(g + 1) * HD],
                              np.float32).reshape(P, 1).copy()
            wv_g = np.ascontiguousarray(
                np.asarray(Wv[:, g * HD:(g + 1) * HD], np.float32))
            bv_g = np.asarray(bv[g * HD:(g + 1) * HD],
                              np.float32).reshape(P, 1).copy()
            wc_g = np.ascontiguousarray(
                np.asarray(Wc[g * H:(g + 1) * H, :], np.float32))
            in_maps.append({
                "qT": qTb, "kT": kTb, "vT": vTb,
                "wq": wq_g, "wk": wk_g, "wv": wv_g, "wc": wc_g,
                "ones": ones, "bqp": bq_g, "bkp": bk_g, "bvp": bv_g,
            })

    _CACHE["in_maps"] = in_maps
    res = run_bass_kernel_spmd(nc, in_maps, list(range(N_CORES)))
    out = np.zeros((B, S, D), np.float32)
    for b in range(B):
        for g in range(G):
            out[b] += res.results[b * G + g]["y"]
    out += np.asarray(bc, np.float32)
    return out
